# revision 1
# baseline (speedup 1.0000x reference)
"""Trainium2 Bass kernel for Conformer-style MultiHeadedAttention (rel-pos, dual bias).

Problem shapes: B=4, T=1024, D=1024, H=16, DK=64, fp32.

Sharding (8 cores, no collectives): core c handles batch b = c//2 and query-row
half th = c%2 (T1 = 512 query rows). Each core computes, fully locally:
  q = query[b, rows] @ Wq + bq            (per-head, duplicated into [qu;qv])
  k = key[b] @ Wk + bk,  v = value[b] @ Wv + bv,  p = pos_emb @ Wp
  S^T[t2,t1] = [k_h;p_h] . [qu_h;qv_h]       (one K=128 matmul per tile)
  E = exp(S^T / 8); sums = 1^T E (M=1 matmul); x^T = v^T E; x^T *= 1/sums
  out[rows] = x @ Wo + bo
Host-side prep (inside kernel(), numpy only): slices per-core shards, transposes
activations to feature-major, duplicates Wq columns per head into [qu|qv] blocks
and folds bq+pos_bias into one per-partition bias table; the k/p projections
evacuate straight into per-head [k_h;p_h] stacked tiles.

All matmul operands are fp16 (full-rate PE streaming, fp32 PSUM accumulate);
biases are added in fp32 from PSUM.
The mask input is all-ones for this problem spec and is accepted but unused.
"""

import os
import sys
from contextlib import ExitStack

import numpy as np

sys.path.insert(0, "/opt/trn_rl_repo")

import concourse.bass as bass  # noqa: E402
import concourse.bacc as bacc  # noqa: E402
import concourse.mybir as mybir  # noqa: E402
import concourse.tile as tile  # noqa: E402

B, T, D, H, DK = 4, 1024, 1024, 16, 64
P = 128
T1 = 512          # query rows per core
KI = D // P       # 8 contraction chunks
N_CORES = 8
F32 = mybir.dt.float32
F16 = mybir.dt.float16
AF = mybir.ActivationFunctionType
OP = mybir.AluOpType
PSUM = bass.MemorySpace.PSUM


def col_slice_ap(dram, c0, width):
    """[D, width] column slice of a [D, N] DRAM tensor as [P, KI, width]."""
    return dram[:, c0:c0 + width].rearrange("(ki p) c -> p ki c", p=P)


def build_program(phases="vqkpao"):
    nc = bacc.Bacc("TRN2", target_bir_lowering=False, debug=False)

    qT_d = nc.dram_tensor("qT", [D, T1], F16, kind="ExternalInput")
    kT_d = nc.dram_tensor("kT", [D, T], F16, kind="ExternalInput")
    vT_d = nc.dram_tensor("vT", [D, T], F16, kind="ExternalInput")
    pT_d = nc.dram_tensor("pT", [D, T], F16, kind="ExternalInput")
    Wq2_d = nc.dram_tensor("Wq2", [D, D], F16, kind="ExternalInput")
    Wk_d = nc.dram_tensor("Wk", [D, D], F16, kind="ExternalInput")
    Wv_d = nc.dram_tensor("Wv", [D, D], F16, kind="ExternalInput")
    Wp_d = nc.dram_tensor("Wp", [D, D], F16, kind="ExternalInput")
    Wo_d = nc.dram_tensor("Wo", [D, D], F16, kind="ExternalInput")
    pb2_d = nc.dram_tensor("pb2", [P, H], F32, kind="ExternalInput")
    bk2_d = nc.dram_tensor("bk2", [P, KI], F32, kind="ExternalInput")
    bv_d = nc.dram_tensor("bv", [1, D], F16, kind="ExternalInput")
    onr_d = nc.dram_tensor("onr", [1, P], F16, kind="ExternalInput")
    bo_d = nc.dram_tensor("bo", [1, D], F16, kind="ExternalInput")
    m5_d = nc.dram_tensor("m5", [P, 1], F32, kind="ExternalInput")
    out_d = nc.dram_tensor("out", [T1, D], F32, kind="ExternalOutput")
    if "D" in phases:
        dbg_v1 = nc.dram_tensor("dbg_v1", [KI, P, H * (DK + 1)], F16,
                                kind="ExternalOutput")
        dbg_qc = nc.dram_tensor("dbg_qc", [H, P, T1], F16, kind="ExternalOutput")
        dbg_kp = nc.dram_tensor("dbg_kp", [H, P, T], F16, kind="ExternalOutput")
        dbg_xT = nc.dram_tensor("dbg_xT", [KI, P, T1], F16, kind="ExternalOutput")

    with tile.TileContext(nc) as tc, ExitStack() as st:
        # ---- persistent pools (live across phases) ----
        v1_p = st.enter_context(tc.tile_pool(name="v1", bufs=KI))
        qcat_p = st.enter_context(tc.tile_pool(name="qcat", bufs=H))
        kp_p = st.enter_context(tc.tile_pool(name="kp", bufs=H))
        xTp = st.enter_context(tc.tile_pool(name="xTsb", bufs=KI))
        const_p = st.enter_context(tc.tile_pool(name="const", bufs=1))

        ones_row = const_p.tile([1, P], F16, tag="ones_row")
        nc.sync.dma_start(ones_row[:], onr_d[:])
        pb2 = const_p.tile([P, H], F32, tag="pb2")
        nc.sync.dma_start(pb2[:], pb2_d[:])
        bk2 = const_p.tile([P, KI], F32, tag="bk2")
        nc.sync.dma_start(bk2[:], bk2_d[:])
        bv_sb = const_p.tile([1, D], F16, tag="bv")
        nc.sync.dma_start(bv_sb[:], bv_d[:])
        bo_sb = const_p.tile([1, D], F16, tag="bo")
        nc.sync.dma_start(bo_sb[:], bo_d[:])
        m5_sb = const_p.tile([P, 1], F32, tag="m5")
        nc.sync.dma_start(m5_sb[:], m5_d[:])

        wo_p = st.enter_context(tc.tile_pool(name="wo", bufs=KI))

        if "v" in phases:
            # ---- phase V: v1[m] = (value @ Wv + bv)[t2-tile m] natural layout ----
            v1 = []
            with tc.tile_pool(name="wv", bufs=KI) as wv_p, \
                 tc.tile_pool(name="vsl", bufs=3) as vsl_p, \
                 tc.tile_pool(name="psv", bufs=3, space=PSUM) as psv_p:
                wv = []
                for ki in range(KI):
                    w = wv_p.tile([P, D], F16, tag="wv")
                    nc.sync.dma_start(w[:], Wv_d[ki * P:(ki + 1) * P, :])
                    wv.append(w)
                for m in range(KI):  # t2 tile
                    vsl = vsl_p.tile([P, KI, P], F16, tag="vsl")
                    nc.sync.dma_start(vsl[:], col_slice_ap(vT_d, m * P, P))
                    ps = psv_p.tile([P, H, DK], F32, tag="psv")
                    for n in range(2):
                        nsl = slice(n * 8, (n + 1) * 8)
                        for ki in range(KI):
                            nc.tensor.matmul(
                                ps[:, nsl, :],
                                vsl[:, ki, :],
                                wv[ki][:, n * T1:(n + 1) * T1],
                                start=(ki == 0), stop=False)
                        # += ones^T @ bv  (broadcast bias over the t2 rows)
                        nc.tensor.matmul(
                            ps[:, nsl, :],
                            ones_row[:, 0:P],
                            bv_sb[:, n * T1:(n + 1) * T1],
                            start=False, stop=True)
                    v1t = v1_p.tile([P, H, DK + 1], F16, tag="v1")
                    nc.vector.tensor_copy(v1t[:, :, 0:DK], ps[:])
                    nc.vector.memset(v1t[:, :, DK:DK + 1], 1.0)
                    v1.append(v1t)

        # activation inputs for Q/K/P, issued after phase V's loads so
        # phase V's weights win the DMA queues
        qin_p = st.enter_context(tc.tile_pool(name="qinp", bufs=KI))
        kin_p = st.enter_context(tc.tile_pool(name="kinp", bufs=KI))
        pin_p = st.enter_context(tc.tile_pool(name="pinp", bufs=KI))
        qin, kin, pin = [], [], []
        for ki in range(KI):
            t = qin_p.tile([P, T1], F16, tag="qin", name=f"qin{ki}")
            nc.sync.dma_start(t[:], qT_d[ki * P:(ki + 1) * P, :])
            qin.append(t)
        for ki in range(KI):
            t = kin_p.tile([P, T], F16, tag="kin", name=f"kin{ki}")
            nc.sync.dma_start(t[:], kT_d[ki * P:(ki + 1) * P, :])
            kin.append(t)
        for ki in range(KI):
            t = pin_p.tile([P, T], F16, tag="pin", name=f"pin{ki}")
            nc.sync.dma_start(t[:], pT_d[ki * P:(ki + 1) * P, :])
            pin.append(t)

        if "q" in phases:
            # ---- phase Q: q computed once per head pair; [qu;qv] built by
            # DVE bias-adds (bias_u half and bias_v half) ----
            qcat = [qcat_p.tile([P, T1], F16, tag="qcat", name=f"qc{h}")
                    for h in range(H)]
            with tc.tile_pool(name="wq", bufs=KI) as wq_p, \
                 tc.tile_pool(name="psq", bufs=4, space=PSUM) as psq_p:
                wq = []
                for ki in range(KI):
                    w = wq_p.tile([P, D], F16, tag="wq", name=f"wq{ki}")
                    nc.sync.dma_start(w[:], Wq2_d[ki * P:(ki + 1) * P, :])
                    wq.append(w)
                for m in range(KI):
                    ps = psq_p.tile([P, T1], F32, tag="psq")
                    for ki in range(KI):
                        nc.tensor.matmul(
                            ps[:],
                            wq[ki][:, m * P:(m + 1) * P],
                            qin[ki][:],
                            start=(ki == 0), stop=(ki == KI - 1))
                    for lo in (0, DK):
                        nc.vector.tensor_scalar_add(
                            qcat[2 * m][lo:lo + DK, :], ps[0:DK, :],
                            pb2[lo:lo + DK, 2 * m:2 * m + 1])
                        nc.vector.tensor_scalar_add(
                            qcat[2 * m + 1][lo:lo + DK, :], ps[DK:P, :],
                            pb2[lo:lo + DK, 2 * m + 1:2 * m + 2])

        if "k" in phases:
            # ---- interleaved per head pair: k-proj, p-proj, then attention
            # for heads {2m, 2m+1}. The pair's exp work (ACT) overlaps the
            # next pair's projection matmuls (PE). ----
            kp = [kp_p.tile([P, T], F16, tag="kp", name=f"kp{h}")
                  for h in range(H)]
            xT = [None] * KI
            with tc.tile_pool(name="wk", bufs=KI) as wk_p, \
                 tc.tile_pool(name="wp", bufs=KI) as wp_p, \
                 tc.tile_pool(name="exps", bufs=2 * KI + 2) as exps_p, \
                 tc.tile_pool(name="rcp", bufs=2) as rcp_p, \
                 tc.tile_pool(name="rbc", bufs=2) as rbc_p, \
                 tc.tile_pool(name="pskp", bufs=2, space=PSUM) as pskp_p, \
                 tc.tile_pool(name="pss", bufs=2, space=PSUM) as pss_p, \
                 tc.tile_pool(name="psx", bufs=1, space=PSUM) as psx_p, \
                 tc.tile_pool(name="psr", bufs=1, space=PSUM) as psr_p:
                wk, wp = [], []
                for ki in range(KI):
                    w = wk_p.tile([P, D], F16, tag="wk", name=f"wk{ki}")
                    nc.sync.dma_start(w[:], Wk_d[ki * P:(ki + 1) * P, :])
                    wk.append(w)
                for ki in range(KI):
                    w = wp_p.tile([P, D], F16, tag="wp", name=f"wp{ki}")
                    nc.sync.dma_start(w[:], Wp_d[ki * P:(ki + 1) * P, :])
                    wp.append(w)
                for m in range(KI):
                    psk = pskp_p.tile([P, T], F32, tag="pskp", name=f"psk{m}")
                    for n in range(2):
                        for ki in range(KI):
                            nc.tensor.matmul(
                                psk[:, n * T1:(n + 1) * T1],
                                wk[ki][:, m * P:(m + 1) * P],
                                kin[ki][:, n * T1:(n + 1) * T1],
                                start=(ki == 0), stop=(ki == KI - 1))
                    nc.vector.tensor_scalar_add(
                        kp[2 * m][0:DK, :], psk[0:DK, :], bk2[0:DK, m:m + 1])
                    nc.vector.tensor_scalar_add(
                        kp[2 * m + 1][0:DK, :], psk[DK:P, :], bk2[DK:P, m:m + 1])
                    psp = pskp_p.tile([P, T], F32, tag="pskp", name=f"psp{m}")
                    for n in range(2):
                        for ki in range(KI):
                            nc.tensor.matmul(
                                psp[:, n * T1:(n + 1) * T1],
                                wp[ki][:, m * P:(m + 1) * P],
                                pin[ki][:, n * T1:(n + 1) * T1],
                                start=(ki == 0), stop=(ki == KI - 1))
                    nc.vector.tensor_copy(kp[2 * m][DK:P, :], psp[0:DK, :])
                    nc.vector.tensor_copy(kp[2 * m + 1][DK:P, :], psp[DK:P, :])

                    for h in (2 * m, 2 * m + 1):
                        # scores^T tiles and exp: one K=128 matmul per t2 tile
                        expS = []
                        for t2t in range(KI):
                            ps = pss_p.tile([P, T1], F32, tag="pss")
                            t2sl = slice(t2t * P, (t2t + 1) * P)
                            nc.tensor.matmul(
                                ps[:],
                                kp[h][:, t2sl],
                                qcat[h][:],
                                start=True, stop=True)
                            es = exps_p.tile([P, T1], F16, tag="expS")
                            # global -5 shift keeps exp/sums inside fp16
                            # range; it cancels exactly in the softmax ratio
                            nc.scalar.activation(es[:], ps[:], AF.Exp,
                                                 scale=1.0 / np.sqrt(DK),
                                                 bias=m5_sb[:])
                            expS.append(es)
                        # x^T = v^T E with the all-ones 65th column giving the
                        # softmax sums in row 64
                        j, hp = h // 2, h % 2
                        psx = psx_p.tile([DK + 1, T1], F32, tag="psx")
                        for t2t in range(KI):
                            nc.tensor.matmul(
                                psx[:],
                                v1[t2t][:, h, 0:DK + 1],
                                expS[t2t][:],
                                start=(t2t == 0), stop=(t2t == KI - 1))
                        # broadcast sums across 64 partitions (K=1 matmul),
                        # then a 64-lane fast reciprocal
                        sums_sb = rcp_p.tile([1, T1], F16, tag="sums_sb")
                        nc.vector.tensor_copy(sums_sb[:], psx[DK:DK + 1, :])
                        psr = psr_p.tile([DK, T1], F32, tag="psr")
                        nc.tensor.matmul(psr[:], ones_row[:, 0:DK],
                                         sums_sb[:], start=True, stop=True)
                        rbc = rbc_p.tile([DK, T1], F32, tag="rbc")
                        nc.vector.reciprocal_approx_fast(rbc[:], psr[:])
                        if hp == 0:
                            xt = xTp.tile([P, T1], F16, tag="xT")
                            xT[j] = xt
                        # DVE re-bases partitions freely: odd heads write the
                        # pair tile's upper half directly.
                        nc.vector.tensor_tensor(
                            xT[j][hp * DK:(hp + 1) * DK, :], psx[0:DK, :],
                            rbc[:], op=OP.mult)

        if "D" in phases:
            for m in range(KI):
                nc.sync.dma_start(dbg_v1[m], v1[m].rearrange("p h c -> p (h c)"))
            for h in range(H):
                nc.sync.dma_start(dbg_qc[h], qcat[h][:])
                nc.sync.dma_start(dbg_kp[h], kp[h][:])
            for ki in range(KI):
                nc.sync.dma_start(dbg_xT[ki], xT[ki][:])

        if "o" in phases:
            # ---- output projection: out = x @ Wo + bo ----
            with tc.tile_pool(name="osb", bufs=2) as osb_p, \
                 tc.tile_pool(name="pso", bufs=4, space=PSUM) as pso_p:
                pso = [pso_p.tile([P, D], F32, tag="pso", name=f"pso{m}")
                       for m in range(T1 // P)]
                wo = []
                for ki in range(KI):
                    w = wo_p.tile([P, D], F16, tag="wo", name=f"wo{ki}")
                    nc.sync.dma_start(w[:], Wo_d[ki * P:(ki + 1) * P, :])
                    wo.append(w)
                for ki in range(KI):
                    w = wo[ki]
                    for m in range(T1 // P):
                        for n in range(2):
                            nsl = slice(n * T1, (n + 1) * T1)
                            nc.tensor.matmul(
                                pso[m][:, nsl],
                                xT[ki][:, m * P:(m + 1) * P],
                                w[:, nsl],
                                start=(ki == 0), stop=False)
                for m in range(T1 // P):
                    for n in range(2):
                        nsl = slice(n * T1, (n + 1) * T1)
                        nc.tensor.matmul(
                            pso[m][:, nsl],
                            ones_row[:, 0:P],
                            bo_sb[:, nsl],
                            start=False, stop=True)
                    ob = osb_p.tile([P, D], F32, tag="osb")
                    nc.scalar.copy(ob[:], pso[m][:])
                    nc.sync.dma_start(out_d[m * P:(m + 1) * P, :], ob[:])

    nc.compile()
    return nc


def prep_core_inputs(query, key, value, pos_emb, Wq, bq, Wk, bk, Wv, bv, Wp,
                     Wo, bo, pos_bias_u, pos_bias_v):
    """Host-side shard + layout prep. Returns list of 8 input dicts."""
    f = np.float32
    query, key, value = np.asarray(query, f), np.asarray(key, f), np.asarray(value, f)
    pos_emb = np.asarray(pos_emb, f)
    Wq, Wk, Wv, Wp, Wo = (np.asarray(a, f) for a in (Wq, Wk, Wv, Wp, Wo))
    bq, bk, bv, bo = (np.asarray(a, f) for a in (bq, bk, bv, bo))
    pbu, pbv = np.asarray(pos_bias_u, f), np.asarray(pos_bias_v, f)

    pb2 = np.empty((P, H), f)
    for h in range(H):
        bu = bq[h * DK:(h + 1) * DK] + pbu[h]
        bvv = bq[h * DK:(h + 1) * DK] + pbv[h]
        pb2[0:DK, h], pb2[DK:P, h] = bu, bvv
    bk2 = np.ascontiguousarray(bk.reshape(KI, P).T)

    h16 = np.float16
    posT = np.ascontiguousarray(pos_emb[0].T).astype(h16)
    shared = dict(Wq2=Wq.astype(h16), Wk=Wk.astype(h16), Wv=Wv.astype(h16),
                  Wp=Wp.astype(h16), Wo=Wo.astype(h16), pb2=pb2, bk2=bk2,
                  bv=bv.reshape(1, D).astype(h16),
                  bo=bo.reshape(1, D).astype(h16), pT=posT,
                  onr=np.ones((1, P), h16), m5=np.full((P, 1), -5.0, f))

    in_maps = []
    kT16 = [np.ascontiguousarray(key[b].T).astype(h16) for b in range(B)]
    vT16 = [np.ascontiguousarray(value[b].T).astype(h16) for b in range(B)]
    for c in range(N_CORES):
        b, th = c // 2, c % 2
        qslice = np.ascontiguousarray(
            query[b].T[:, th * T1:(th + 1) * T1]).astype(h16)
        in_maps.append(dict(qT=qslice, kT=kT16[b], vT=vT16[b], **shared))
    return in_maps


def assemble_output(results):
    out = np.empty((B, T, D), np.float32)
    for c in range(N_CORES):
        b, th = c // 2, c % 2
        out[b, th * T1:(th + 1) * T1, :] = results[c]["out"]
    return out


_NC_CACHE = None


def get_program():
    global _NC_CACHE
    if _NC_CACHE is None:
        _NC_CACHE = build_program()
    return _NC_CACHE


def kernel(**inputs) -> np.ndarray:
    from concourse.bass_utils import run_bass_kernel_spmd

    inputs.pop("mask", None)  # all-ones for this problem; softmax unaffected
    in_maps = prep_core_inputs(**inputs)
    nc = get_program()
    res = run_bass_kernel_spmd(nc, in_maps, list(range(N_CORES)))
    return assemble_output(res.results)


if __name__ == "__main__":
    get_program()
    print("program built OK")



# revision 14
# speedup vs baseline: 1.3088x; 1.3088x over previous
"""Trainium2 Bass kernel for Conformer-style MultiHeadedAttention (rel-pos, dual bias).

Problem shapes: B=4, T=1024, D=1024, H=16, DK=64, fp32.

Sharding (8 cores, no collectives): core c handles batch b = c//2 and head-half
hh = c%2 (8 heads, all T=1024 query rows). Each core computes its heads'
Q/K/P/V projections and attention, then a PARTIAL output projection
x_local @ Wo[hh-block]; the host sums the two partials per batch and adds the
combined bias. Softmax algebra used on device:

  S = (q+bu)·k + (q+bv)·p  =  q·(k+p) + [bu·k + bv·p]  (+ per-row consts)
    - per-(t1) constants (q·bk etc.) cancel in softmax -> dropped
    - eb = exp([bu·k + bv·p]/8) depends only on (key, pos, weights); it is
      precomputed on the host as a small [t2, head] table and folded into the
      v operand (and its all-ones 65th column), so the big exp over scores
      has no bias term -> batched [128, 1024] ACT ops, one per (head, t2-tile
      pair)
  x = attn @ (v0 + bv) = attn@v0 + bv  ->  bv@Wo + bo added on host.

All matmuls are fp16 (fp8 tested ~10x too lossy: e4m3's 3.6% per-element RMS
error passes straight through to GEMM outputs). Scores matmuls contract K=64
and run row-tiled: both heads of a pair execute concurrently in the top/bottom
halves of the PE array (partitions 0:64 / 64:128). The kpsum trick (k+p summed
in PSUM by accumulating the K and P projections into one bank) halves scores
work vs. the dual-bias formulation. mask is all-ones for this problem, unused.

Emission is software-pipelined: the ACT-paced scores+exp stream of pair m is
interleaved with attn@v of pair m-1, K+P projection of pair m+1, and the
one-time V/Q phases, so the PE always has independent queued work.
"""

import sys

import numpy as np

sys.path.insert(0, "/opt/trn_rl_repo")

import concourse.bass as bass  # noqa: E402
import concourse.bacc as bacc  # noqa: E402
import concourse.mybir as mybir  # noqa: E402
import concourse.tile as tile  # noqa: E402

B, T, D, H, DK = 4, 1024, 1024, 16, 64
P = 128
HC = 8            # heads per core
NPAIR = 4         # head pairs per core
KI = 8            # contraction chunks of 128
N_CORES = 8
F32 = mybir.dt.float32
F16 = mybir.dt.float16
AF = mybir.ActivationFunctionType
OP = mybir.AluOpType
PSUM = bass.MemorySpace.PSUM


def build_program():
    nc = bacc.Bacc("TRN2", target_bir_lowering=False, debug=False)

    # activations, feature-major [D, T]
    qT_d = nc.dram_tensor("qT", [D, T], F16, kind="ExternalInput")
    kT_d = nc.dram_tensor("kT", [D, T], F16, kind="ExternalInput")
    vT_d = nc.dram_tensor("vT", [D, T], F16, kind="ExternalInput")
    pT_d = nc.dram_tensor("pT", [D, T], F16, kind="ExternalInput")
    # weight column-blocks for this half: [D, 512]; Wo rows-block [512, D]
    Wq_d = nc.dram_tensor("Wq", [D, 512], F16, kind="ExternalInput")
    Wk_d = nc.dram_tensor("Wk", [D, 512], F16, kind="ExternalInput")
    Wp_d = nc.dram_tensor("Wp", [D, 512], F16, kind="ExternalInput")
    Wv_d = nc.dram_tensor("Wv", [D, 512], F16, kind="ExternalInput")
    Wo_d = nc.dram_tensor("Wo", [512, D], F16, kind="ExternalInput")
    ebc_d = nc.dram_tensor("ebc", [P, 64], F16, kind="ExternalInput")
    bq2_d = nc.dram_tensor("bq2", [P, NPAIR], F32, kind="ExternalInput")
    onr_d = nc.dram_tensor("onr", [1, P], F16, kind="ExternalInput")
    m4_d = nc.dram_tensor("m4", [P, 1], F32, kind="ExternalInput")
    out_d = nc.dram_tensor("out", [T, D], F16, kind="ExternalOutput")

    with tile.TileContext(nc) as tc:
        with tc.tile_pool(name="const", bufs=1) as const_p, \
             tc.tile_pool(name="wgt", bufs=KI) as wgt_p, \
             tc.tile_pool(name="wo", bufs=4) as wo_p, \
             tc.tile_pool(name="acts", bufs=KI) as act_p, \
             tc.tile_pool(name="qsb", bufs=NPAIR) as qsb_p, \
             tc.tile_pool(name="kpsum", bufs=NPAIR) as kp_p, \
             tc.tile_pool(name="v1", bufs=KI) as v1_p, \
             tc.tile_pool(name="es", bufs=12) as es_p, \
             tc.tile_pool(name="xT", bufs=2) as xT_p, \
             tc.tile_pool(name="sums", bufs=2) as sums_p, \
             tc.tile_pool(name="rbc", bufs=2) as rbc_p, \
             tc.tile_pool(name="osb", bufs=2) as osb_p, \
             tc.tile_pool(name="ps1", bufs=4, space=PSUM) as ps1_p, \
             tc.tile_pool(name="ps2", bufs=2, space=PSUM) as ps2_p:

            ebc = const_p.tile([P, 8, HC], F16, tag="ebc")
            nc.sync.dma_start(ebc[:], ebc_d[:].rearrange("p (t h) -> p t h",
                                                         t=8))
            bq2 = const_p.tile([P, NPAIR], F32, tag="bq2")
            nc.sync.dma_start(bq2[:], bq2_d[:])
            onr = const_p.tile([1, P], F16, tag="onr")
            nc.sync.dma_start(onr[:], onr_d[:])
            m4 = const_p.tile([P, 1], F32, tag="m4")
            nc.sync.dma_start(m4[:], m4_d[:])

            def load_chunks(name, dram, cols, pool):
                ts = []
                for ki in range(dram.shape[0] // 128):
                    t = pool.tile([128, cols], F16, tag=name,
                                  name=f"{name}{ki}")
                    nc.sync.dma_start(t[:], dram[ki * 128:(ki + 1) * 128, :])
                    ts.append(t)
                return ts

            # DMA issue order = order of first use; K/P interleaved by chunk
            # so kp(0)'s accumulation can start while the rest streams in.
            kin, pin, wk, wp = [], [], [], []
            for ki in range(KI):
                for lst, dram, nm in ((wk, Wk_d, "wk"), (kin, kT_d, "kin"),
                                      (wp, Wp_d, "wp"), (pin, pT_d, "pin")):
                    cols = 512 if nm in ("wk", "wp") else T
                    pool = wgt_p if nm in ("wk", "wp") else act_p
                    t = pool.tile([128, cols], F16, tag=nm, name=f"{nm}{ki}")
                    nc.sync.dma_start(
                        t[:], dram[ki * 128:(ki + 1) * 128, :])
                    lst.append(t)
            qin = load_chunks("qin", qT_d, T, act_p)
            wq = load_chunks("wq", Wq_d, 512, wgt_p)
            vin = load_chunks("vin", vT_d, T, act_p)
            wv = load_chunks("wv", Wv_d, 512, wgt_p)
            wo = load_chunks("wo", Wo_d, D, wo_p)

            kpsum = [None] * NPAIR
            qsb = [None] * NPAIR
            v1 = [None] * KI
            es = {}
            xT = [xT_p.tile([128, 2, 1024], F16, tag="xT", name=f"xT{kj}")
                  for kj in range(2)]

            def emit_kp_half(m, n):
                if kpsum[m] is None:
                    kpsum[m] = kp_p.tile([128, 1024], F16, tag="kpsum",
                                         name=f"kp{m}")
                psk = ps1_p.tile([128, 512], F32, tag="ps1", name=f"psk{m}{n}")
                for src, w in ((kin, wk), (pin, wp)):
                    for ki in range(KI):
                        nc.tensor.matmul(
                            psk[:],
                            w[ki][:, m * 128:(m + 1) * 128],
                            src[ki][:, n * 512:(n + 1) * 512],
                            start=(src is kin and ki == 0),
                            stop=(src is pin and ki == KI - 1))
                nc.vector.tensor_copy(
                    kpsum[m][:, n * 512:(n + 1) * 512], psk[:])

            def emit_q(m):
                qt = qsb_p.tile([128, 1024], F16, tag="qsb", name=f"q{m}")
                qsb[m] = qt
                for n in range(2):
                    psq = ps1_p.tile([128, 512], F32, tag="ps1",
                                     name=f"psq{m}{n}")
                    for ki in range(KI):
                        nc.tensor.matmul(
                            psq[:],
                            wq[ki][:, m * 128:(m + 1) * 128],
                            qin[ki][:, n * 512:(n + 1) * 512],
                            start=(ki == 0), stop=(ki == KI - 1))
                    nc.vector.tensor_scalar_add(
                        qt[:, n * 512:(n + 1) * 512], psq[:], bq2[:, m:m + 1])

            def emit_v(t2t):
                # v1[t2t][t2, h, 0:64] = v0 * eb ; [..., 64] = eb
                v1[t2t] = v1_p.tile([128, HC, 66], F16, tag="v1",
                                    name=f"v1_{t2t}")
                psv = ps1_p.tile([128, 512], F32, tag="ps1", name=f"psv{t2t}")
                for ki in range(KI):
                    nc.tensor.matmul(
                        psv[:],
                        vin[ki][:, t2t * 128:(t2t + 1) * 128],
                        wv[ki][:],
                        start=(ki == 0), stop=(ki == KI - 1))
                ebb = ebc[:, t2t, :].unsqueeze(2).broadcast_to([128, HC, 64])
                nc.vector.tensor_tensor(
                    v1[t2t][:, :, 0:64],
                    psv[:].rearrange("p (h c) -> p h c", h=HC),
                    ebb, op=OP.mult)
                nc.vector.tensor_copy(
                    v1[t2t][:, :, 64:65], ebc[:, t2t, :].unsqueeze(2))

            def emit_scores_tile(m, t2t):
                # fp16 K=64 row-tiled scores for both heads + batched exp
                j, i = t2t // 2, t2t % 2
                if i == 0:
                    for hp in range(2):
                        es[(m, hp, j)] = es_p.tile(
                            [128, 2, 1024], F16, tag="es", name=f"es{m}{hp}{j}")
                pss = [ps2_p.tile([128, 1024], F32, tag="ps2",
                                  name=f"pss{m}{t2t}{hp}") for hp in range(2)]
                for hp in range(2):
                    rs = slice(hp * 64, hp * 64 + 64)
                    for n in range(2):
                        nc.tensor.matmul(
                            pss[hp][:, n * 512:(n + 1) * 512],
                            kpsum[m][rs, t2t * 128:(t2t + 1) * 128],
                            qsb[m][rs, n * 512:(n + 1) * 512],
                            start=True, stop=True)
                for hp in range(2):
                    # -4 shift keeps exp inside fp16 range for ~8-sigma
                    # logits; it cancels exactly in the softmax ratio
                    nc.scalar.activation(
                        es[(m, hp, j)][:, i, :], pss[hp][:], AF.Exp,
                        scale=0.125, bias=m4[:])

            def emit_attn_head(m, hp):
                h = 2 * m + hp
                psx = [ps1_p.tile([128, 512], F32, tag="ps1",
                                  name=f"psx{h}{n}") for n in range(2)]
                for n in range(2):
                    for t2t in range(KI):
                        nc.tensor.matmul(
                            psx[n][0:65, :],
                            v1[t2t][:, h, 0:65],
                            es[(m, hp, t2t // 2)][:, t2t % 2,
                                                  n * 512:(n + 1) * 512],
                            start=(t2t == 0), stop=(t2t == KI - 1))
                sums = sums_p.tile([1, 1024], F16, tag="sums", name=f"sm{h}")
                nc.vector.tensor_copy(sums[:, 0:512], psx[0][64:65, :])
                nc.vector.tensor_copy(sums[:, 512:1024], psx[1][64:65, :])
                kj, i = h // 4, (h // 2) % 2
                rs = slice((h % 2) * 64, (h % 2) * 64 + 64)
                for n in range(2):
                    psr = ps1_p.tile([128, 512], F32, tag="ps1",
                                     name=f"psr{h}{n}")
                    nc.tensor.matmul(
                        psr[0:64, :], onr[:, 0:64],
                        sums[:, n * 512:(n + 1) * 512],
                        start=True, stop=True)
                    rbc = rbc_p.tile([64, 512], F32, tag="rbc")
                    nc.vector.reciprocal_approx_fast(rbc[:], psr[0:64, :])
                    nc.vector.tensor_tensor(
                        xT[kj][rs, i, n * 512:(n + 1) * 512],
                        psx[n][0:64, :], rbc[:], op=OP.mult)

            # ---- software-pipelined emission ----
            emit_kp_half(0, 0)
            emit_kp_half(0, 1)
            emit_q(0)
            for m in range(NPAIR):
                emit_scores_tile(m, 0)
                emit_scores_tile(m, 1)
                if m > 0:
                    emit_attn_head(m - 1, 0)
                else:
                    emit_v(0)
                    emit_v(1)
                emit_scores_tile(m, 2)
                emit_scores_tile(m, 3)
                if m < NPAIR - 1:
                    emit_kp_half(m + 1, 0)
                if m == 0:
                    emit_v(2)
                    emit_v(3)
                    emit_q(1)
                emit_scores_tile(m, 4)
                emit_scores_tile(m, 5)
                if m > 0:
                    emit_attn_head(m - 1, 1)
                else:
                    emit_v(4)
                    emit_v(5)
                    emit_q(2)
                emit_scores_tile(m, 6)
                emit_scores_tile(m, 7)
                if m < NPAIR - 1:
                    emit_kp_half(m + 1, 1)
                if m == 0:
                    emit_v(6)
                    emit_v(7)
                    emit_q(3)
            emit_attn_head(NPAIR - 1, 0)
            emit_attn_head(NPAIR - 1, 1)

            # ---- output projection (partial): out = xT.T @ Wo[hh-block]
            for m in range(8):
                ob = osb_p.tile([128, 1024], F16, tag="osb", name=f"ob{m}")
                for n in range(2):
                    pso = ps1_p.tile([128, 512], F32, tag="ps1",
                                     name=f"pso{m}{n}")
                    for kj in range(4):
                        nc.tensor.matmul(
                            pso[:],
                            xT[kj // 2][:, kj % 2, m * 128:(m + 1) * 128],
                            wo[kj][:, n * 512:(n + 1) * 512],
                            start=(kj == 0), stop=(kj == 3))
                    nc.vector.tensor_copy(ob[:, n * 512:(n + 1) * 512], pso[:])
                nc.sync.dma_start(out_d[m * 128:(m + 1) * 128, :], ob[:])

    nc.compile()
    return nc


def prep_core_inputs(query, key, value, pos_emb, Wq, bq, Wk, bk, Wv, bv, Wp,
                     Wo, bo, pos_bias_u, pos_bias_v):
    """Host-side shard + layout prep. Returns (list of 8 input dicts, bo2)."""
    f, h16 = np.float32, np.float16
    query, key, value = (np.asarray(a, f) for a in (query, key, value))
    pos_emb = np.asarray(pos_emb, f)
    Wq, Wk, Wv, Wp, Wo = (np.asarray(a, f) for a in (Wq, Wk, Wv, Wp, Wo))
    bq, bk, bv, bo = (np.asarray(a, f) for a in (bq, bk, bv, bo))
    pbu, pbv = np.asarray(pos_bias_u, f), np.asarray(pos_bias_v, f)

    bo2 = bo + bv @ Wo  # combined output bias (x = attn@v0 + bv exactly)

    pT = np.ascontiguousarray(pos_emb[0].T).astype(h16)
    kT = [np.ascontiguousarray(key[b].T).astype(h16) for b in range(B)]
    vT = [np.ascontiguousarray(value[b].T).astype(h16) for b in range(B)]
    qT = [np.ascontiguousarray(query[b].T).astype(h16) for b in range(B)]

    # eb[t2, gh] = exp((bu_gh . k_gh[t2] + bv_gh . p_gh[t2]) / 8), per batch
    k0 = [key[b] @ Wk for b in range(B)]       # no bk: constant in t2-softmax
    p0 = pos_emb[0] @ Wp
    eb = np.empty((B, T, H), f)
    for gh in range(H):
        blk = slice(gh * DK, (gh + 1) * DK)
        for b in range(B):
            eb[b, :, gh] = np.exp(
                (k0[b][:, blk] @ pbu[gh] + p0[:, blk] @ pbv[gh]) / 8.0)

    shared = dict(pT=pT, onr=np.ones((1, P), h16),
                  m4=np.full((P, 1), -4.0, f))
    halves = []
    for hh in range(2):
        cs = slice(hh * 512, (hh + 1) * 512)
        bq2 = np.ascontiguousarray(bq[cs].reshape(NPAIR, 128).T).astype(f)
        halves.append(dict(
            Wq=Wq[:, cs].astype(h16), Wk=Wk[:, cs].astype(h16),
            Wp=Wp[:, cs].astype(h16), Wv=Wv[:, cs].astype(h16),
            Wo=np.ascontiguousarray(Wo[cs, :]).astype(h16), bq2=bq2))

    in_maps = []
    for c in range(N_CORES):
        b, hh = c // 2, c % 2
        # ebc [p, t2t*8 + h]: eb for t2 = t2t*128 + p, head hh*8+h
        ebcore = eb[b][:, hh * HC:(hh + 1) * HC].reshape(8, 128, HC)
        ebcore = np.ascontiguousarray(
            ebcore.transpose(1, 0, 2).reshape(128, 64)).astype(h16)
        in_maps.append(dict(qT=qT[b], kT=kT[b], vT=vT[b], ebc=ebcore,
                            **halves[hh], **shared))
    return in_maps, bo2


def assemble_output(results, bo2):
    out = np.empty((B, T, D), np.float32)
    for b in range(B):
        out[b] = (results[2 * b]["out"].astype(np.float32)
                  + results[2 * b + 1]["out"].astype(np.float32) + bo2)
    return out


_NC_CACHE = None


def get_program():
    global _NC_CACHE
    if _NC_CACHE is None:
        _NC_CACHE = build_program()
    return _NC_CACHE


def kernel(**inputs) -> np.ndarray:
    from concourse.bass_utils import run_bass_kernel_spmd

    inputs.pop("mask", None)  # all-ones for this problem; softmax unaffected
    in_maps, bo2 = prep_core_inputs(**inputs)
    nc = get_program()
    res = run_bass_kernel_spmd(nc, in_maps, list(range(N_CORES)))
    return assemble_output(res.results, bo2)


if __name__ == "__main__":
    get_program()
    print("program built OK")
